# revision 37
# baseline (speedup 1.0000x reference)
"""CrossAttention Trainium2 kernel (nn_CrossAttention_28544352649420).

Full-input contract: kernel(**inputs) takes the unsharded arrays
  inputA [8,2048,1024] f32, inputB [8,2048,1024] f32,
  maskA [8,2048] f32, maskB [8,2048] f32, W [1024,1024] f32, b [1024] f32
and returns (cvA [8,2048,1024], cvB [8,2048,1024]) matching

  projA  = inputA @ W + b
  scores = projA @ inputB^T, masked_fill(maskA x maskB == 0, -1e9)
  attnA  = softmax(scores, axis=1); attnB = softmax(scores, axis=2)
  cvA    = attnA^T @ inputA;        cvB = attnB @ inputB

Sharding: batch dim across the 8 NeuronCores (data parallel, SPMD —
one batch element per core; every core holds the full W).

Per-core schedule (B=1, La=Lb=2048, Da=Db=1024):
  Phase A: projAT[e,l] = (inputA @ W)^T via PE-transposed inputA strips,
           fp32 matmuls, streamed to a DRAM scratch.
  Phase B: inputBT[e,m] hi/lo bf16 (PE transpose) + inputB bf16 resident.
  Pass 1 (16 l-strips, software-pipelined): S strip via split-bf16
           (hi/lo) matmuls, mask via min-masks, rowmax (exact,
           per-partition), E_B = exp(S - rowmax) bf16 (+ fused denom),
           per-strip PE transpose of E_B -> cvB strip =
           E_B^T @ inputB_bf / denom. Fully-masked scores stream to a
           DRAM scratch while the exact per-column max accumulates on
           the idle gpsimd engine (partition_all_reduce + running max).
  Phase C (16 m-chunks, software-pipelined): load the masked-score
           column slab, E_A = exp(s - colmax) bf16, cvA chunk =
           E_A^T @ inputA_bf / colsum (ones-vector matmul colsum).
"""
import sys

sys.path.insert(0, "/opt/trn_rl_repo")

import numpy as np
from contextlib import ExitStack

import concourse.bass as bass
import concourse.tile as tile
from concourse import bacc
from concourse import mybir
from concourse import bass_isa
from concourse.bass_utils import run_bass_kernel_spmd
from concourse.masks import make_identity

F32 = mybir.dt.float32
F32R = mybir.dt.float32r
BF16 = mybir.dt.bfloat16
MIN = mybir.AluOpType.min
MULT = mybir.AluOpType.mult
ADD = mybir.AluOpType.add
SUB = mybir.AluOpType.subtract
MAXOP = mybir.AluOpType.max
EXP = mybir.ActivationFunctionType.Exp
X = mybir.AxisListType.X

B, L, D = 8, 2048, 1024
NS = L // 128  # 16 strips
KC = D // 128  # 8 contraction chunks
BIG = 1.0e30
NEG = -1.0e9
SHIFT = 25.0

_CACHE = {}


def build():
    nc = bacc.Bacc(trn_type="TRN2")

    inputA = nc.declare_dram_parameter("inputA", [L, D], F32, isOutput=False)
    inputB = nc.declare_dram_parameter("inputB", [L, D], F32, isOutput=False)
    maskA = nc.declare_dram_parameter("maskA", [L, 1], F32, isOutput=False)
    maskB = nc.declare_dram_parameter("maskB", [1, L], F32, isOutput=False)
    Wp = nc.declare_dram_parameter("W", [D, D], F32, isOutput=False)
    bp = nc.declare_dram_parameter("b", [D, 1], F32, isOutput=False)
    cvA = nc.declare_dram_parameter("cvA", [L, D], F32, isOutput=True)
    cvB = nc.declare_dram_parameter("cvB", [L, D], F32, isOutput=True)

    projATh_d = nc.dram_tensor("projATh_d", [D, L], BF16)  # (inputA @ W)^T hi
    projATl_d = nc.dram_tensor("projATl_d", [D, L], BF16)  # lo residual
    TMIN_d = nc.dram_tensor("TMIN_d", [L, L], F32)  # fully-masked scores

    with tile.TileContext(nc) as tc, ExitStack() as ctx:
        glob = ctx.enter_context(tc.tile_pool(name="glob", bufs=1))

        ident = glob.tile([128, 128], F32)
        make_identity(nc, ident)
        ident_bf = glob.tile([128, 128], BF16)
        make_identity(nc, ident_bf)
        ones_bf = glob.tile([128, 1], BF16)
        nc.vector.memset(ones_bf, 1.0)


        b_t = glob.tile([128, KC], F32)
        maA = glob.tile([128, NS], F32)
        maA_min = glob.tile([128, NS], F32)  # 1 -> +BIG, 0 -> NEG
        MBb = glob.tile([128, L], F32)
        a_bf = glob.tile([128, NS, D], BF16)  # [l-part, lc, e] for phase C
        cmax_full = glob.tile([128, L], F32)  # running colmax of masked scores
        nc.vector.memset(cmax_full, -3.0e38)

        def load_small_globals():
            nc.sync.dma_start(
                out=b_t, in_=bp[:].rearrange("(c p) o -> p (c o)", p=128)
            )
            nc.sync.dma_start(
                out=maA, in_=maskA[:].rearrange("(s p) o -> p (s o)", p=128)
            )
            nc.vector.tensor_scalar(
                out=maA_min, in0=maA, scalar1=BIG - NEG, scalar2=NEG,
                op0=MULT, op1=ADD,
            )


        # ---------------- Phase A: projAT (software-pipelined pairs) ----------------
        with tc.tile_pool(name="pa", bufs=1) as pa, \
             tc.tile_pool(name="pa_ps", bufs=1, space="PSUM") as pa_ps:
            at2s = {}

            def a_stage1(pair):
                at2_hi = pa.tile([128, KC, 256], BF16, tag="at2_hi", bufs=2)
                at2_lo = pa.tile([128, KC, 256], BF16, tag="at2_lo", bufs=2)
                for s in range(2):
                    stripA = pa.tile([128, D], F32, tag="stripA", bufs=3)
                    nc.sync.dma_start(
                        out=stripA,
                        in_=inputA[(2 * pair + s) * 128:(2 * pair + s + 1) * 128, :],
                    )
                    nc.scalar.copy(out=a_bf[:, 2 * pair + s, :], in_=stripA)
                    for g in range(2):
                        tp = pa_ps.tile([128, 4, 128], F32, tag="tp", bufs=2)
                        for j in range(4):
                            dc = g * 4 + j
                            nc.tensor.transpose(
                                tp[:, j, :], stripA[:, dc * 128:(dc + 1) * 128], ident
                            )
                        hi_sl = at2_hi[:, g * 4:(g + 1) * 4, s * 128:(s + 1) * 128]
                        nc.scalar.copy(out=hi_sl, in_=tp)
                        nc.vector.tensor_tensor(
                            out=at2_lo[:, g * 4:(g + 1) * 4, s * 128:(s + 1) * 128],
                            in0=tp, in1=hi_sl, op=SUB,
                        )
                at2s[pair] = (at2_hi, at2_lo)

            def a_stage2(pair):
                at2_hi, at2_lo = at2s.pop(pair)
                for ec in range(KC):
                    pj = pa_ps.tile([128, 256], F32, tag="pj", bufs=4)
                    wterms = [(w_hi, at2_hi), (w_hi, at2_lo), (w_lo, at2_hi)]
                    for t, (wt, at) in enumerate(wterms):
                        for dc in range(KC):
                            nc.tensor.matmul(
                                pj,
                                wt[:, dc, ec * 128:(ec + 1) * 128],
                                at[:, dc, :],
                                start=(t == 0 and dc == 0),
                                stop=(t == 2 and dc == KC - 1),
                            )
                    pjs_hi = pa.tile([128, 256], BF16, tag="pjs_hi", bufs=3)
                    nc.vector.tensor_scalar(
                        out=pjs_hi, in0=pj, scalar1=b_t[:, ec:ec + 1], scalar2=None,
                        op0=ADD,
                    )
                    pjs_lo = pa.tile([128, 256], BF16, tag="pjs_lo", bufs=3)
                    nc.vector.scalar_tensor_tensor(
                        out=pjs_lo, in0=pj, scalar=b_t[:, ec:ec + 1], in1=pjs_hi,
                        op0=ADD, op1=SUB,
                    )
                    nc.sync.dma_start(
                        out=projATh_d[:].rearrange("(c p) l -> p c l", p=128)[
                            :, ec, pair * 256:(pair + 1) * 256
                        ],
                        in_=pjs_hi,
                    )
                    nc.sync.dma_start(
                        out=projATl_d[:].rearrange("(c p) l -> p c l", p=128)[
                            :, ec, pair * 256:(pair + 1) * 256
                        ],
                        in_=pjs_lo,
                    )

            a_stage1(0)
            load_small_globals()
            w_f = pa.tile([128, KC, D], F32)
            nc.sync.dma_start(
                out=w_f, in_=Wp[:].rearrange("(c p) e -> p c e", p=128)
            )
            w_hi = pa.tile([128, KC, D], BF16)
            nc.gpsimd.tensor_copy(out=w_hi, in_=w_f)
            w_lo = pa.tile([128, KC, D], BF16)
            nc.vector.tensor_tensor(out=w_lo, in0=w_f, in1=w_hi, op=SUB)
            for pair in range(NS // 2):
                if pair + 1 < NS // 2:
                    a_stage1(pair + 1)
                a_stage2(pair)

        # ------- Phase B (inputBT f32r + inputB bf16) and Pass 1 share a pool
        with tc.tile_pool(name="p1", bufs=1) as p1, \
             tc.tile_pool(name="p1_ps", bufs=1, space="PSUM") as p1_ps:
            nc.sync.dma_start(
                out=MBb,
                in_=maskB[:].rearrange("o n -> (o n)").partition_broadcast(128),
            )
            nc.vector.tensor_scalar(
                out=MBb, in0=MBb, scalar1=BIG - NEG, scalar2=NEG,
                op0=MULT, op1=ADD,
            )
            bt_hi = p1.tile([128, KC, L], BF16)  # [e-part, ec, m]
            bt_lo = p1.tile([128, KC, L], BF16)
            b_bf = p1.tile([128, NS, D], BF16)  # [m-part, mc, e]
            for i in range(NS):
                stripB = p1.tile([128, D], F32, tag="stripB", bufs=2)
                nc.sync.dma_start(out=stripB, in_=inputB[i * 128:(i + 1) * 128, :])
                nc.gpsimd.tensor_copy(out=b_bf[:, i, :], in_=stripB)
                for g in range(2):
                    tp2 = p1_ps.tile([128, 4, 128], F32, tag="tp", bufs=2)
                    for j in range(4):
                        ec = g * 4 + j
                        nc.tensor.transpose(
                            tp2[:, j, :], stripB[:, ec * 128:(ec + 1) * 128], ident
                        )
                    hi_sl = bt_hi[:, g * 4:(g + 1) * 4, i * 128:(i + 1) * 128]
                    nc.scalar.copy(out=hi_sl, in_=tp2)
                    nc.vector.tensor_tensor(
                        out=bt_lo[:, g * 4:(g + 1) * 4, i * 128:(i + 1) * 128],
                        in0=tp2, in1=hi_sl, op=SUB,
                    )
            prev = {}

            def stage1(i):
                pAT_hi = p1.tile([128, KC, 128], BF16, tag="pAT_hi", bufs=2)
                nc.sync.dma_start(
                    out=pAT_hi,
                    in_=projATh_d[:].rearrange("(c p) l -> p c l", p=128)[
                        :, :, i * 128:(i + 1) * 128
                    ],
                )
                pAT_lo = p1.tile([128, KC, 128], BF16, tag="pAT_lo", bufs=2)
                nc.sync.dma_start(
                    out=pAT_lo,
                    in_=projATl_d[:].rearrange("(c p) l -> p c l", p=128)[
                        :, :, i * 128:(i + 1) * 128
                    ],
                )
                smask = p1.tile([128, L], F32, tag="smask", bufs=2)
                for h in range(2):
                    sps = p1_ps.tile([128, 1024], F32, tag="sps", bufs=2)
                    for nb in range(2):
                        msl = slice(h * 1024 + nb * 512, h * 1024 + (nb + 1) * 512)
                        terms = [(pAT_hi, bt_hi), (pAT_hi, bt_lo), (pAT_lo, bt_hi)]
                        for t, (lt, rt) in enumerate(terms):
                            for ec in range(KC):
                                nc.tensor.matmul(
                                    sps[:, nb * 512:(nb + 1) * 512],
                                    lt[:, ec, :],
                                    rt[:, ec, msl],
                                    start=(t == 0 and ec == 0),
                                    stop=(t == 2 and ec == KC - 1),
                                )
                    nc.vector.tensor_tensor(
                        out=smask[:, h * 1024:(h + 1) * 1024],
                        in0=sps,
                        in1=MBb[:, h * 1024:(h + 1) * 1024],
                        op=MIN,
                    )
                negrm = p1.tile([128, 1], F32, tag="negrm", bufs=2)
                nc.vector.reduce_max(out=negrm, in_=smask, axis=X, negate=True)
                biasB = p1.tile([128, 1], F32, tag="biasB", bufs=2)
                nc.vector.tensor_tensor(
                    out=biasB, in0=negrm, in1=maA[:, i:i + 1], op=MULT
                )
                eb = p1.tile([128, L], BF16, tag="eb", bufs=2)
                denomB = p1.tile([128, 1], F32, tag="denomB", bufs=2)
                nc.scalar.activation(
                    out=eb, in_=smask, func=EXP,
                    bias=biasB, scale=maA[:, i:i + 1], accum_out=denomB,
                )
                # fully-masked scores (A-mask applied too) -> DRAM for phase C;
                # overwrites smask in place after the E_B exp has consumed it
                nc.vector.tensor_scalar_min(smask, smask, maA_min[:, i:i + 1])
                nc.sync.dma_start(out=TMIN_d[i * 128:(i + 1) * 128, :], in_=smask)
                # per-strip column max -> running colmax (idle gpsimd engine)
                for h in range(2):
                    ar = p1.tile([128, 1024], F32, tag="ar", bufs=1)
                    hsl = slice(h * 1024, (h + 1) * 1024)
                    nc.gpsimd.partition_all_reduce(
                        ar, smask[:, hsl], channels=128,
                        reduce_op=bass_isa.ReduceOp.max,
                    )
                    nc.vector.tensor_tensor(
                        out=cmax_full[:, hsl], in0=cmax_full[:, hsl], in1=ar, op=MAXOP
                    )


                prev[i] = (eb, denomB)

            def stage2(i):
                eb, denomB = prev.pop(i)
                # E_B^T tiles and cvB strip
                ebt = p1.tile([128, NS, 128], BF16, tag="ebt", bufs=2)
                for g in range(2):
                    tp3 = p1_ps.tile([128, 8, 128], BF16, tag="tp", bufs=2)
                    for j in range(8):
                        mc = g * 8 + j
                        nc.tensor.transpose(
                            tp3[:, j, :], eb[:, mc * 128:(mc + 1) * 128], ident_bf
                        )
                    nc.scalar.copy(out=ebt[:, g * 8:(g + 1) * 8, :], in_=tp3)
                ups = p1_ps.tile([128, D], F32, tag="ups", bufs=1)
                for nb in range(2):
                    for mc in range(NS):
                        nc.tensor.matmul(
                            ups[:, nb * 512:(nb + 1) * 512],
                            ebt[:, mc, :],
                            b_bf[:, mc, nb * 512:(nb + 1) * 512],
                            start=(mc == 0),
                            stop=(mc == NS - 1),
                        )
                rden = p1.tile([128, 1], F32, tag="rden", bufs=2)
                nc.vector.reciprocal(out=rden, in_=denomB)
                cvb_sb = p1.tile([128, D], F32, tag="cvb_sb", bufs=2)
                nc.vector.tensor_scalar(
                    out=cvb_sb, in0=ups, scalar1=rden, scalar2=None, op0=MULT
                )
                nc.sync.dma_start(out=cvB[i * 128:(i + 1) * 128, :], in_=cvb_sb)


            for i in range(NS):
                stage1(i)
                if i > 0:
                    stage2(i - 1)
            stage2(NS - 1)

        # ---------------- Phase C: cvA per m-chunk ----------------
        with tc.tile_pool(name="pc", bufs=1) as pc, \
             tc.tile_pool(name="pc_ps", bufs=1, space="PSUM") as pc_ps:
            cprev = {}
            subs_last = {}

            def c_stage1(j):
                # tmin column slab: [l-part, lc, m] for 128 columns m
                slab = pc.tile([128, NS, 128], F32, tag="slab", bufs=6)
                nc.sync.dma_start(
                    out=slab,
                    in_=TMIN_d[:, j * 128:(j + 1) * 128].rearrange(
                        "(c p) m -> p c m", p=128
                    ),
                )
                # E_A tiles = exp(tmin - colmax), bf16 (colmax from pass 1)
                cmb = cmax_full[:, j * 128:(j + 1) * 128].unsqueeze(1).broadcast_to(
                    (128, 4, 128)
                )
                for q in range(4):
                    sub_inst = nc.vector.tensor_tensor(
                        out=slab[:, q * 4:(q + 1) * 4, :],
                        in0=slab[:, q * 4:(q + 1) * 4, :], in1=cmb, op=SUB,
                    )
                subs_last[j] = sub_inst
                ea_t = pc.tile([128, NS, 128], BF16, tag="ea_t", bufs=6)
                nc.scalar.activation(out=ea_t, in_=slab, func=EXP)

                cprev[j] = ea_t

            def c_stage2(j):
                ea_t = cprev.pop(j)
                aps = pc_ps.tile([128, D], F32, tag="aps", bufs=3)
                csum = pc_ps.tile([128, 1], F32, tag="csum", bufs=2)
                for lc in range(NS):
                    nc.tensor.matmul(
                        csum, ea_t[:, lc, :], ones_bf,
                        start=(lc == 0), stop=(lc == NS - 1),
                    )
                for nb in range(2):
                    for lc in range(NS):
                        nc.tensor.matmul(
                            aps[:, nb * 512:(nb + 1) * 512],
                            ea_t[:, lc, :],
                            a_bf[:, lc, nb * 512:(nb + 1) * 512],
                            start=(lc == 0),
                            stop=(lc == NS - 1),
                        )
                rcs = pc.tile([128, 1], F32, tag="rcs", bufs=2)
                recip_inst = nc.vector.reciprocal(out=rcs, in_=csum)
                if j + 2 in subs_last:
                    tile.add_dep_helper(
                        recip_inst.ins, subs_last[j + 2].ins, sync=False,
                        reason="keep cvA evict after next chunks' subs on DVE",
                    )
                cva_sb = pc.tile([128, D], F32, tag="cva_sb", bufs=2)
                nc.vector.tensor_scalar(
                    out=cva_sb, in0=aps, scalar1=rcs, scalar2=None, op0=MULT
                )
                nc.sync.dma_start(out=cvA[j * 128:(j + 1) * 128, :], in_=cva_sb)


            c_stage1(0)
            c_stage1(1)
            for j in range(NS):
                if j + 2 < NS:
                    c_stage1(j + 2)
                c_stage2(j)
    if not nc.is_finalized():
        nc.finalize()
    return nc


def run(inputs, trace=False, trace_kwargs=None):
    if "nc" not in _CACHE:
        _CACHE["nc"] = build()
    nc = _CACHE["nc"]
    in_maps = []
    for i in range(B):
        in_maps.append({
            "inputA": np.ascontiguousarray(inputs["inputA"][i], dtype=np.float32),
            "inputB": np.ascontiguousarray(inputs["inputB"][i], dtype=np.float32),
            "maskA": np.ascontiguousarray(
                inputs["maskA"][i], dtype=np.float32).reshape(L, 1),
            "maskB": np.ascontiguousarray(
                inputs["maskB"][i], dtype=np.float32).reshape(1, L),
            "W": np.ascontiguousarray(inputs["W"], dtype=np.float32),
            "b": np.ascontiguousarray(inputs["b"], dtype=np.float32).reshape(D, 1),
        })
    try:
        res = run_bass_kernel_spmd(
            nc, in_maps, core_ids=list(range(B)), trace=trace,
            **(trace_kwargs or {}),
        )
    except ModuleNotFoundError:
        res = run_bass_kernel_spmd(nc, in_maps, core_ids=list(range(B)), trace=False)
    cva = np.stack([res.results[i]["cvA"] for i in range(B)]).astype(np.float32)
    cvb = np.stack([res.results[i]["cvB"] for i in range(B)]).astype(np.float32)
    return (cva, cvb), res


def kernel(**inputs):
    (cva, cvb), _ = run(inputs, trace=False)
    return cva, cvb


# revision 38
# speedup vs baseline: 1.0050x; 1.0050x over previous
"""CrossAttention Trainium2 kernel (nn_CrossAttention_28544352649420).

Full-input contract: kernel(**inputs) takes the unsharded arrays
  inputA [8,2048,1024] f32, inputB [8,2048,1024] f32,
  maskA [8,2048] f32, maskB [8,2048] f32, W [1024,1024] f32, b [1024] f32
and returns (cvA [8,2048,1024], cvB [8,2048,1024]) matching

  projA  = inputA @ W + b
  scores = projA @ inputB^T, masked_fill(maskA x maskB == 0, -1e9)
  attnA  = softmax(scores, axis=1); attnB = softmax(scores, axis=2)
  cvA    = attnA^T @ inputA;        cvB = attnB @ inputB

Sharding: batch dim across the 8 NeuronCores (data parallel, SPMD —
one batch element per core; every core holds the full W).

Per-core schedule (B=1, La=Lb=2048, Da=Db=1024):
  Phase A: projAT[e,l] = (inputA @ W)^T via PE-transposed inputA strips,
           fp32 matmuls, streamed to a DRAM scratch.
  Phase B: inputBT[e,m] hi/lo bf16 (PE transpose) + inputB bf16 resident.
  Pass 1 (16 l-strips, software-pipelined): S strip via split-bf16
           (hi/lo) matmuls, mask via min-masks, rowmax (exact,
           per-partition), E_B = exp(S - rowmax) bf16 (+ fused denom),
           per-strip PE transpose of E_B -> cvB strip =
           E_B^T @ inputB_bf / denom. Fully-masked scores stream to a
           DRAM scratch while the exact per-column max accumulates on
           the idle gpsimd engine (partition_all_reduce + running max).
  Phase C (16 m-chunks, software-pipelined): load the masked-score
           column slab, E_A = exp(s - colmax) bf16, cvA chunk =
           E_A^T @ inputA_bf / colsum (ones-vector matmul colsum).
"""
import sys

sys.path.insert(0, "/opt/trn_rl_repo")

import numpy as np
from contextlib import ExitStack

import concourse.bass as bass
import concourse.tile as tile
from concourse import bacc
from concourse import mybir
from concourse import bass_isa
from concourse.bass_utils import run_bass_kernel_spmd
from concourse.masks import make_identity

F32 = mybir.dt.float32
F32R = mybir.dt.float32r
BF16 = mybir.dt.bfloat16
MIN = mybir.AluOpType.min
MULT = mybir.AluOpType.mult
ADD = mybir.AluOpType.add
SUB = mybir.AluOpType.subtract
MAXOP = mybir.AluOpType.max
EXP = mybir.ActivationFunctionType.Exp
X = mybir.AxisListType.X

B, L, D = 8, 2048, 1024
NS = L // 128  # 16 strips
KC = D // 128  # 8 contraction chunks
BIG = 1.0e30
NEG = -1.0e9
SHIFT = 25.0

_CACHE = {}


def build():
    nc = bacc.Bacc(trn_type="TRN2")

    inputA = nc.declare_dram_parameter("inputA", [L, D], F32, isOutput=False)
    inputB = nc.declare_dram_parameter("inputB", [L, D], F32, isOutput=False)
    maskA = nc.declare_dram_parameter("maskA", [L, 1], F32, isOutput=False)
    maskB = nc.declare_dram_parameter("maskB", [1, L], F32, isOutput=False)
    Wp = nc.declare_dram_parameter("W", [D, D], F32, isOutput=False)
    bp = nc.declare_dram_parameter("b", [D, 1], F32, isOutput=False)
    cvA = nc.declare_dram_parameter("cvA", [L, D], F32, isOutput=True)
    cvB = nc.declare_dram_parameter("cvB", [L, D], F32, isOutput=True)

    projATh_d = nc.dram_tensor("projATh_d", [D, L], BF16)  # (inputA @ W)^T hi
    projATl_d = nc.dram_tensor("projATl_d", [D, L], BF16)  # lo residual
    TMIN_d = nc.dram_tensor("TMIN_d", [L, L], F32)  # fully-masked scores

    with tile.TileContext(nc) as tc, ExitStack() as ctx:
        glob = ctx.enter_context(tc.tile_pool(name="glob", bufs=1))

        ident = glob.tile([128, 128], F32)
        make_identity(nc, ident)
        ident_bf = glob.tile([128, 128], BF16)
        make_identity(nc, ident_bf)
        ones_bf = glob.tile([128, 1], BF16)
        nc.vector.memset(ones_bf, 1.0)


        b_t = glob.tile([128, KC], F32)
        maA = glob.tile([128, NS], F32)
        maA_min = glob.tile([128, NS], F32)  # 1 -> +BIG, 0 -> NEG
        MBb = glob.tile([128, L], F32)
        a_bf = glob.tile([128, NS, D], BF16)  # [l-part, lc, e] for phase C
        cmax_full = glob.tile([128, L], F32)  # running colmax of masked scores
        nc.vector.memset(cmax_full, -3.0e38)

        def load_small_globals():
            nc.sync.dma_start(
                out=b_t, in_=bp[:].rearrange("(c p) o -> p (c o)", p=128)
            )
            nc.sync.dma_start(
                out=maA, in_=maskA[:].rearrange("(s p) o -> p (s o)", p=128)
            )
            nc.vector.tensor_scalar(
                out=maA_min, in0=maA, scalar1=BIG - NEG, scalar2=NEG,
                op0=MULT, op1=ADD,
            )


        # ---------------- Phase A: projAT (software-pipelined pairs) ----------------
        with tc.tile_pool(name="pa", bufs=1) as pa, \
             tc.tile_pool(name="pa_ps", bufs=1, space="PSUM") as pa_ps:
            at2s = {}

            def a_stage1(pair):
                at2_hi = pa.tile([128, KC, 256], BF16, tag="at2_hi", bufs=2)
                at2_lo = pa.tile([128, KC, 256], BF16, tag="at2_lo", bufs=2)
                for s in range(2):
                    stripA = pa.tile([128, D], F32, tag="stripA", bufs=3)
                    nc.sync.dma_start(
                        out=stripA,
                        in_=inputA[(2 * pair + s) * 128:(2 * pair + s + 1) * 128, :],
                    )
                    nc.scalar.copy(out=a_bf[:, 2 * pair + s, :], in_=stripA)
                    for g in range(2):
                        tp = pa_ps.tile([128, 4, 128], F32, tag="tp", bufs=2)
                        for j in range(4):
                            dc = g * 4 + j
                            nc.tensor.transpose(
                                tp[:, j, :], stripA[:, dc * 128:(dc + 1) * 128], ident
                            )
                        hi_sl = at2_hi[:, g * 4:(g + 1) * 4, s * 128:(s + 1) * 128]
                        nc.scalar.copy(out=hi_sl, in_=tp)
                        nc.vector.tensor_tensor(
                            out=at2_lo[:, g * 4:(g + 1) * 4, s * 128:(s + 1) * 128],
                            in0=tp, in1=hi_sl, op=SUB,
                        )
                at2s[pair] = (at2_hi, at2_lo)

            def a_stage2(pair):
                at2_hi, at2_lo = at2s.pop(pair)
                for ec in range(KC):
                    pj = pa_ps.tile([128, 256], F32, tag="pj", bufs=4)
                    wterms = [(w_hi, at2_hi), (w_hi, at2_lo), (w_lo, at2_hi)]
                    for t, (wt, at) in enumerate(wterms):
                        for dc in range(KC):
                            nc.tensor.matmul(
                                pj,
                                wt[:, dc, ec * 128:(ec + 1) * 128],
                                at[:, dc, :],
                                start=(t == 0 and dc == 0),
                                stop=(t == 2 and dc == KC - 1),
                            )
                    pjs_hi = pa.tile([128, 256], BF16, tag="pjs_hi", bufs=3)
                    nc.vector.tensor_scalar(
                        out=pjs_hi, in0=pj, scalar1=b_t[:, ec:ec + 1], scalar2=None,
                        op0=ADD,
                    )
                    pjs_lo = pa.tile([128, 256], BF16, tag="pjs_lo", bufs=3)
                    nc.vector.scalar_tensor_tensor(
                        out=pjs_lo, in0=pj, scalar=b_t[:, ec:ec + 1], in1=pjs_hi,
                        op0=ADD, op1=SUB,
                    )
                    nc.sync.dma_start(
                        out=projATh_d[:].rearrange("(c p) l -> p c l", p=128)[
                            :, ec, pair * 256:(pair + 1) * 256
                        ],
                        in_=pjs_hi,
                    )
                    nc.sync.dma_start(
                        out=projATl_d[:].rearrange("(c p) l -> p c l", p=128)[
                            :, ec, pair * 256:(pair + 1) * 256
                        ],
                        in_=pjs_lo,
                    )

            a_stage1(0)
            load_small_globals()
            w_f = pa.tile([128, KC, D], F32)
            nc.sync.dma_start(
                out=w_f, in_=Wp[:].rearrange("(c p) e -> p c e", p=128)
            )
            w_hi = pa.tile([128, KC, D], BF16)
            nc.gpsimd.tensor_copy(out=w_hi, in_=w_f)
            w_lo = pa.tile([128, KC, D], BF16)
            nc.vector.tensor_tensor(out=w_lo, in0=w_f, in1=w_hi, op=SUB)
            for pair in range(NS // 2):
                if pair + 1 < NS // 2:
                    a_stage1(pair + 1)
                a_stage2(pair)

        # ------- Phase B (inputBT f32r + inputB bf16) and Pass 1 share a pool
        with tc.tile_pool(name="p1", bufs=1) as p1, \
             tc.tile_pool(name="p1_ps", bufs=1, space="PSUM") as p1_ps:
            nc.sync.dma_start(
                out=MBb,
                in_=maskB[:].rearrange("o n -> (o n)").partition_broadcast(128),
            )
            nc.vector.tensor_scalar(
                out=MBb, in0=MBb, scalar1=BIG - NEG, scalar2=NEG,
                op0=MULT, op1=ADD,
            )
            bt_hi = p1.tile([128, KC, L], BF16)  # [e-part, ec, m]
            bt_lo = p1.tile([128, KC, L], BF16)
            b_bf = p1.tile([128, NS, D], BF16)  # [m-part, mc, e]
            for i in range(NS):
                stripB = p1.tile([128, D], F32, tag="stripB", bufs=2)
                nc.sync.dma_start(out=stripB, in_=inputB[i * 128:(i + 1) * 128, :])
                nc.gpsimd.tensor_copy(out=b_bf[:, i, :], in_=stripB)
                for g in range(2):
                    tp2 = p1_ps.tile([128, 4, 128], F32, tag="tp", bufs=2)
                    for j in range(4):
                        ec = g * 4 + j
                        nc.tensor.transpose(
                            tp2[:, j, :], stripB[:, ec * 128:(ec + 1) * 128], ident
                        )
                    hi_sl = bt_hi[:, g * 4:(g + 1) * 4, i * 128:(i + 1) * 128]
                    nc.scalar.copy(out=hi_sl, in_=tp2)
                    nc.vector.tensor_tensor(
                        out=bt_lo[:, g * 4:(g + 1) * 4, i * 128:(i + 1) * 128],
                        in0=tp2, in1=hi_sl, op=SUB,
                    )
            prev = {}

            def stage1(i):
                pAT_hi = p1.tile([128, KC, 128], BF16, tag="pAT_hi", bufs=2)
                nc.sync.dma_start(
                    out=pAT_hi,
                    in_=projATh_d[:].rearrange("(c p) l -> p c l", p=128)[
                        :, :, i * 128:(i + 1) * 128
                    ],
                )
                pAT_lo = p1.tile([128, KC, 128], BF16, tag="pAT_lo", bufs=2)
                nc.sync.dma_start(
                    out=pAT_lo,
                    in_=projATl_d[:].rearrange("(c p) l -> p c l", p=128)[
                        :, :, i * 128:(i + 1) * 128
                    ],
                )
                smask = p1.tile([128, L], F32, tag="smask", bufs=2)
                for h in range(2):
                    sps = p1_ps.tile([128, 1024], F32, tag="sps", bufs=2)
                    for nb in range(2):
                        msl = slice(h * 1024 + nb * 512, h * 1024 + (nb + 1) * 512)
                        terms = [(pAT_hi, bt_hi), (pAT_hi, bt_lo), (pAT_lo, bt_hi)]
                        for t, (lt, rt) in enumerate(terms):
                            for ec in range(KC):
                                nc.tensor.matmul(
                                    sps[:, nb * 512:(nb + 1) * 512],
                                    lt[:, ec, :],
                                    rt[:, ec, msl],
                                    start=(t == 0 and ec == 0),
                                    stop=(t == 2 and ec == KC - 1),
                                )
                    nc.vector.tensor_tensor(
                        out=smask[:, h * 1024:(h + 1) * 1024],
                        in0=sps,
                        in1=MBb[:, h * 1024:(h + 1) * 1024],
                        op=MIN,
                    )
                negrm = p1.tile([128, 1], F32, tag="negrm", bufs=2)
                nc.vector.reduce_max(out=negrm, in_=smask, axis=X, negate=True)
                biasB = p1.tile([128, 1], F32, tag="biasB", bufs=2)
                nc.vector.tensor_tensor(
                    out=biasB, in0=negrm, in1=maA[:, i:i + 1], op=MULT
                )
                eb = p1.tile([128, L], BF16, tag="eb", bufs=2)
                denomB = p1.tile([128, 1], F32, tag="denomB", bufs=2)
                nc.scalar.activation(
                    out=eb, in_=smask, func=EXP,
                    bias=biasB, scale=maA[:, i:i + 1], accum_out=denomB,
                )
                # fully-masked scores (A-mask applied too) -> DRAM for phase C;
                # overwrites smask in place after the E_B exp has consumed it
                nc.vector.tensor_scalar_min(smask, smask, maA_min[:, i:i + 1])
                nc.sync.dma_start(out=TMIN_d[i * 128:(i + 1) * 128, :], in_=smask)
                # per-strip column max -> running colmax (idle gpsimd engine)
                for h in range(2):
                    ar = p1.tile([128, 1024], F32, tag="ar", bufs=1)
                    hsl = slice(h * 1024, (h + 1) * 1024)
                    nc.gpsimd.partition_all_reduce(
                        ar, smask[:, hsl], channels=128,
                        reduce_op=bass_isa.ReduceOp.max,
                    )
                    nc.vector.tensor_tensor(
                        out=cmax_full[:, hsl], in0=cmax_full[:, hsl], in1=ar, op=MAXOP
                    )


                prev[i] = (eb, denomB)

            def stage2(i):
                eb, denomB = prev.pop(i)
                # E_B^T tiles and cvB strip
                ebt = p1.tile([128, NS, 128], BF16, tag="ebt", bufs=2)
                for g in range(2):
                    tp3 = p1_ps.tile([128, 8, 128], BF16, tag="tp", bufs=2)
                    for j in range(8):
                        mc = g * 8 + j
                        nc.tensor.transpose(
                            tp3[:, j, :], eb[:, mc * 128:(mc + 1) * 128], ident_bf
                        )
                    nc.scalar.copy(out=ebt[:, g * 8:(g + 1) * 8, :], in_=tp3)
                ups = p1_ps.tile([128, D], F32, tag="ups", bufs=1)
                for nb in range(2):
                    for mc in range(NS):
                        nc.tensor.matmul(
                            ups[:, nb * 512:(nb + 1) * 512],
                            ebt[:, mc, :],
                            b_bf[:, mc, nb * 512:(nb + 1) * 512],
                            start=(mc == 0),
                            stop=(mc == NS - 1),
                        )
                rden = p1.tile([128, 1], F32, tag="rden", bufs=2)
                nc.vector.reciprocal(out=rden, in_=denomB)
                cvb_sb = p1.tile([128, D], F32, tag="cvb_sb", bufs=2)
                nc.vector.tensor_scalar(
                    out=cvb_sb, in0=ups, scalar1=rden, scalar2=None, op0=MULT
                )
                nc.sync.dma_start(out=cvB[i * 128:(i + 1) * 128, :], in_=cvb_sb)


            for i in range(NS):
                stage1(i)
                if i > 0:
                    stage2(i - 1)
            stage2(NS - 1)

        # ---------------- Phase C: cvA per m-chunk ----------------
        with tc.tile_pool(name="pc", bufs=1) as pc, \
             tc.tile_pool(name="pc_ps", bufs=1, space="PSUM") as pc_ps:
            cprev = {}
            subs_last = {}

            def c_stage1(j):
                # tmin column slab: [l-part, lc, m] for 128 columns m
                slab = pc.tile([128, NS, 128], F32, tag="slab", bufs=6)
                nc.sync.dma_start(
                    out=slab,
                    in_=TMIN_d[:, j * 128:(j + 1) * 128].rearrange(
                        "(c p) m -> p c m", p=128
                    ),
                )
                # E_A tiles = exp(tmin - colmax), bf16 (colmax from pass 1)
                cmb = cmax_full[:, j * 128:(j + 1) * 128].unsqueeze(1).broadcast_to(
                    (128, 4, 128)
                )
                for q in range(4):
                    sub_inst = nc.vector.tensor_tensor(
                        out=slab[:, q * 4:(q + 1) * 4, :],
                        in0=slab[:, q * 4:(q + 1) * 4, :], in1=cmb, op=SUB,
                    )
                subs_last[j] = sub_inst
                ea_t = pc.tile([128, NS, 128], BF16, tag="ea_t", bufs=6)
                for hh in range(2):
                    nc.scalar.activation(
                        out=ea_t[:, hh * 8:(hh + 1) * 8, :],
                        in_=slab[:, hh * 8:(hh + 1) * 8, :], func=EXP,
                    )

                cprev[j] = ea_t

            def c_stage2(j):
                ea_t = cprev.pop(j)
                aps = pc_ps.tile([128, D], F32, tag="aps", bufs=3)
                csum = pc_ps.tile([128, 1], F32, tag="csum", bufs=2)
                for lc in range(NS):
                    nc.tensor.matmul(
                        csum, ea_t[:, lc, :], ones_bf,
                        start=(lc == 0), stop=(lc == NS - 1),
                    )
                for nb in range(2):
                    for lc in range(NS):
                        nc.tensor.matmul(
                            aps[:, nb * 512:(nb + 1) * 512],
                            ea_t[:, lc, :],
                            a_bf[:, lc, nb * 512:(nb + 1) * 512],
                            start=(lc == 0),
                            stop=(lc == NS - 1),
                        )
                rcs = pc.tile([128, 1], F32, tag="rcs", bufs=2)
                recip_inst = nc.vector.reciprocal(out=rcs, in_=csum)
                if j + 2 in subs_last:
                    tile.add_dep_helper(
                        recip_inst.ins, subs_last[j + 2].ins, sync=False,
                        reason="keep cvA evict after next chunks' subs on DVE",
                    )
                cva_sb = pc.tile([128, D], F32, tag="cva_sb", bufs=2)
                nc.vector.tensor_scalar(
                    out=cva_sb, in0=aps, scalar1=rcs, scalar2=None, op0=MULT
                )
                nc.sync.dma_start(out=cvA[j * 128:(j + 1) * 128, :], in_=cva_sb)


            c_stage1(0)
            c_stage1(1)
            for j in range(NS):
                if j + 2 < NS:
                    c_stage1(j + 2)
                c_stage2(j)
    if not nc.is_finalized():
        nc.finalize()
    return nc


def run(inputs, trace=False, trace_kwargs=None):
    if "nc" not in _CACHE:
        _CACHE["nc"] = build()
    nc = _CACHE["nc"]
    in_maps = []
    for i in range(B):
        in_maps.append({
            "inputA": np.ascontiguousarray(inputs["inputA"][i], dtype=np.float32),
            "inputB": np.ascontiguousarray(inputs["inputB"][i], dtype=np.float32),
            "maskA": np.ascontiguousarray(
                inputs["maskA"][i], dtype=np.float32).reshape(L, 1),
            "maskB": np.ascontiguousarray(
                inputs["maskB"][i], dtype=np.float32).reshape(1, L),
            "W": np.ascontiguousarray(inputs["W"], dtype=np.float32),
            "b": np.ascontiguousarray(inputs["b"], dtype=np.float32).reshape(D, 1),
        })
    try:
        res = run_bass_kernel_spmd(
            nc, in_maps, core_ids=list(range(B)), trace=trace,
            **(trace_kwargs or {}),
        )
    except ModuleNotFoundError:
        res = run_bass_kernel_spmd(nc, in_maps, core_ids=list(range(B)), trace=False)
    cva = np.stack([res.results[i]["cvA"] for i in range(B)]).astype(np.float32)
    cvb = np.stack([res.results[i]["cvB"] for i in range(B)]).astype(np.float32)
    return (cva, cvb), res


def kernel(**inputs):
    (cva, cvb), _ = run(inputs, trace=False)
    return cva, cvb


# revision 39
# speedup vs baseline: 1.0064x; 1.0014x over previous
"""CrossAttention Trainium2 kernel (nn_CrossAttention_28544352649420).

Full-input contract: kernel(**inputs) takes the unsharded arrays
  inputA [8,2048,1024] f32, inputB [8,2048,1024] f32,
  maskA [8,2048] f32, maskB [8,2048] f32, W [1024,1024] f32, b [1024] f32
and returns (cvA [8,2048,1024], cvB [8,2048,1024]) matching

  projA  = inputA @ W + b
  scores = projA @ inputB^T, masked_fill(maskA x maskB == 0, -1e9)
  attnA  = softmax(scores, axis=1); attnB = softmax(scores, axis=2)
  cvA    = attnA^T @ inputA;        cvB = attnB @ inputB

Sharding: batch dim across the 8 NeuronCores (data parallel, SPMD —
one batch element per core; every core holds the full W).

Per-core schedule (B=1, La=Lb=2048, Da=Db=1024):
  Phase A: projAT[e,l] = (inputA @ W)^T via PE-transposed inputA strips,
           fp32 matmuls, streamed to a DRAM scratch.
  Phase B: inputBT[e,m] hi/lo bf16 (PE transpose) + inputB bf16 resident.
  Pass 1 (16 l-strips, software-pipelined): S strip via split-bf16
           (hi/lo) matmuls, mask via min-masks, rowmax (exact,
           per-partition), E_B = exp(S - rowmax) bf16 (+ fused denom),
           per-strip PE transpose of E_B -> cvB strip =
           E_B^T @ inputB_bf / denom. Fully-masked scores stream to a
           DRAM scratch while the exact per-column max accumulates on
           the idle gpsimd engine (partition_all_reduce + running max).
  Phase C (16 m-chunks, software-pipelined): load the masked-score
           column slab, E_A = exp(s - colmax) bf16, cvA chunk =
           E_A^T @ inputA_bf / colsum (ones-vector matmul colsum).
"""
import sys

sys.path.insert(0, "/opt/trn_rl_repo")

import numpy as np
from contextlib import ExitStack

import concourse.bass as bass
import concourse.tile as tile
from concourse import bacc
from concourse import mybir
from concourse import bass_isa
from concourse.bass_utils import run_bass_kernel_spmd
from concourse.masks import make_identity

F32 = mybir.dt.float32
F32R = mybir.dt.float32r
BF16 = mybir.dt.bfloat16
MIN = mybir.AluOpType.min
MULT = mybir.AluOpType.mult
ADD = mybir.AluOpType.add
SUB = mybir.AluOpType.subtract
MAXOP = mybir.AluOpType.max
EXP = mybir.ActivationFunctionType.Exp
X = mybir.AxisListType.X

B, L, D = 8, 2048, 1024
NS = L // 128  # 16 strips
KC = D // 128  # 8 contraction chunks
BIG = 1.0e30
NEG = -1.0e9
SHIFT = 25.0

_CACHE = {}


def build():
    nc = bacc.Bacc(trn_type="TRN2")

    inputA = nc.declare_dram_parameter("inputA", [L, D], F32, isOutput=False)
    inputB = nc.declare_dram_parameter("inputB", [L, D], F32, isOutput=False)
    maskA = nc.declare_dram_parameter("maskA", [L, 1], F32, isOutput=False)
    maskB = nc.declare_dram_parameter("maskB", [1, L], F32, isOutput=False)
    Wp = nc.declare_dram_parameter("W", [D, D], F32, isOutput=False)
    bp = nc.declare_dram_parameter("b", [D, 1], F32, isOutput=False)
    cvA = nc.declare_dram_parameter("cvA", [L, D], F32, isOutput=True)
    cvB = nc.declare_dram_parameter("cvB", [L, D], F32, isOutput=True)

    projATh_d = nc.dram_tensor("projATh_d", [D, L], BF16)  # (inputA @ W)^T hi
    projATl_d = nc.dram_tensor("projATl_d", [D, L], BF16)  # lo residual
    TMIN_d = nc.dram_tensor("TMIN_d", [L, L], F32)  # fully-masked scores

    with tile.TileContext(nc) as tc, ExitStack() as ctx:
        glob = ctx.enter_context(tc.tile_pool(name="glob", bufs=1))

        ident = glob.tile([128, 128], F32)
        make_identity(nc, ident)
        ident_bf = glob.tile([128, 128], BF16)
        make_identity(nc, ident_bf)
        ones_bf = glob.tile([128, 1], BF16)
        nc.vector.memset(ones_bf, 1.0)


        b_t = glob.tile([128, KC], F32)
        maA = glob.tile([128, NS], F32)
        maA_min = glob.tile([128, NS], F32)  # 1 -> +BIG, 0 -> NEG
        MBb = glob.tile([128, L], F32)
        a_bf = glob.tile([128, NS, D], BF16)  # [l-part, lc, e] for phase C
        cmax_full = glob.tile([128, L], F32)  # running colmax of masked scores
        nc.vector.memset(cmax_full, -3.0e38)

        def load_small_globals():
            nc.sync.dma_start(
                out=b_t, in_=bp[:].rearrange("(c p) o -> p (c o)", p=128)
            )
            nc.sync.dma_start(
                out=maA, in_=maskA[:].rearrange("(s p) o -> p (s o)", p=128)
            )
            nc.vector.tensor_scalar(
                out=maA_min, in0=maA, scalar1=BIG - NEG, scalar2=NEG,
                op0=MULT, op1=ADD,
            )


        # ---------------- Phase A: projAT (software-pipelined pairs) ----------------
        with tc.tile_pool(name="pa", bufs=1) as pa, \
             tc.tile_pool(name="pa_ps", bufs=1, space="PSUM") as pa_ps:
            at2s = {}

            def a_stage1(pair):
                at2_hi = pa.tile([128, KC, 256], BF16, tag="at2_hi", bufs=2)
                at2_lo = pa.tile([128, KC, 256], BF16, tag="at2_lo", bufs=2)
                for s in range(2):
                    stripA = pa.tile([128, D], F32, tag="stripA", bufs=3)
                    nc.sync.dma_start(
                        out=stripA,
                        in_=inputA[(2 * pair + s) * 128:(2 * pair + s + 1) * 128, :],
                    )
                    nc.scalar.copy(out=a_bf[:, 2 * pair + s, :], in_=stripA)
                    for g in range(2):
                        tp = pa_ps.tile([128, 4, 128], F32, tag="tp", bufs=2)
                        for j in range(4):
                            dc = g * 4 + j
                            nc.tensor.transpose(
                                tp[:, j, :], stripA[:, dc * 128:(dc + 1) * 128], ident
                            )
                        hi_sl = at2_hi[:, g * 4:(g + 1) * 4, s * 128:(s + 1) * 128]
                        nc.scalar.copy(out=hi_sl, in_=tp)
                        nc.vector.tensor_tensor(
                            out=at2_lo[:, g * 4:(g + 1) * 4, s * 128:(s + 1) * 128],
                            in0=tp, in1=hi_sl, op=SUB,
                        )
                at2s[pair] = (at2_hi, at2_lo)

            def a_stage2(pair):
                at2_hi, at2_lo = at2s.pop(pair)
                for ec in range(KC):
                    pj = pa_ps.tile([128, 256], F32, tag="pj", bufs=4)
                    wterms = [(w_hi, at2_hi), (w_hi, at2_lo), (w_lo, at2_hi)]
                    for t, (wt, at) in enumerate(wterms):
                        for dc in range(KC):
                            nc.tensor.matmul(
                                pj,
                                wt[:, dc, ec * 128:(ec + 1) * 128],
                                at[:, dc, :],
                                start=(t == 0 and dc == 0),
                                stop=(t == 2 and dc == KC - 1),
                            )
                    pjs_hi = pa.tile([128, 256], BF16, tag="pjs_hi", bufs=3)
                    nc.vector.tensor_scalar(
                        out=pjs_hi, in0=pj, scalar1=b_t[:, ec:ec + 1], scalar2=None,
                        op0=ADD,
                    )
                    pjs_lo = pa.tile([128, 256], BF16, tag="pjs_lo", bufs=3)
                    nc.vector.scalar_tensor_tensor(
                        out=pjs_lo, in0=pj, scalar=b_t[:, ec:ec + 1], in1=pjs_hi,
                        op0=ADD, op1=SUB,
                    )
                    nc.sync.dma_start(
                        out=projATh_d[:].rearrange("(c p) l -> p c l", p=128)[
                            :, ec, pair * 256:(pair + 1) * 256
                        ],
                        in_=pjs_hi,
                    )
                    nc.sync.dma_start(
                        out=projATl_d[:].rearrange("(c p) l -> p c l", p=128)[
                            :, ec, pair * 256:(pair + 1) * 256
                        ],
                        in_=pjs_lo,
                    )

            a_stage1(0)
            load_small_globals()
            w_f = pa.tile([128, KC, D], F32)
            nc.sync.dma_start(
                out=w_f, in_=Wp[:].rearrange("(c p) e -> p c e", p=128)
            )
            w_hi = pa.tile([128, KC, D], BF16)
            nc.gpsimd.tensor_copy(out=w_hi, in_=w_f)
            w_lo = pa.tile([128, KC, D], BF16)
            nc.vector.tensor_tensor(out=w_lo, in0=w_f, in1=w_hi, op=SUB)
            for pair in range(NS // 2):
                if pair + 1 < NS // 2:
                    a_stage1(pair + 1)
                a_stage2(pair)

        # ------- Phase B (inputBT f32r + inputB bf16) and Pass 1 share a pool
        with tc.tile_pool(name="p1", bufs=1) as p1, \
             tc.tile_pool(name="p1_ps", bufs=1, space="PSUM") as p1_ps:
            nc.sync.dma_start(
                out=MBb,
                in_=maskB[:].rearrange("o n -> (o n)").partition_broadcast(128),
            )
            nc.vector.tensor_scalar(
                out=MBb, in0=MBb, scalar1=BIG - NEG, scalar2=NEG,
                op0=MULT, op1=ADD,
            )
            bt_hi = p1.tile([128, KC, L], BF16)  # [e-part, ec, m]
            bt_lo = p1.tile([128, KC, L], BF16)
            b_bf = p1.tile([128, NS, D], BF16)  # [m-part, mc, e]
            for i in range(NS):
                stripB = p1.tile([128, D], F32, tag="stripB", bufs=2)
                nc.sync.dma_start(out=stripB, in_=inputB[i * 128:(i + 1) * 128, :])
                nc.gpsimd.tensor_copy(out=b_bf[:, i, :], in_=stripB)
                for g in range(2):
                    tp2 = p1_ps.tile([128, 4, 128], F32, tag="tp", bufs=2)
                    for j in range(4):
                        ec = g * 4 + j
                        nc.tensor.transpose(
                            tp2[:, j, :], stripB[:, ec * 128:(ec + 1) * 128], ident
                        )
                    hi_sl = bt_hi[:, g * 4:(g + 1) * 4, i * 128:(i + 1) * 128]
                    nc.scalar.copy(out=hi_sl, in_=tp2)
                    nc.vector.tensor_tensor(
                        out=bt_lo[:, g * 4:(g + 1) * 4, i * 128:(i + 1) * 128],
                        in0=tp2, in1=hi_sl, op=SUB,
                    )
            prev = {}

            def stage1(i):
                pAT_hi = p1.tile([128, KC, 128], BF16, tag="pAT_hi", bufs=2)
                nc.sync.dma_start(
                    out=pAT_hi,
                    in_=projATh_d[:].rearrange("(c p) l -> p c l", p=128)[
                        :, :, i * 128:(i + 1) * 128
                    ],
                )
                pAT_lo = p1.tile([128, KC, 128], BF16, tag="pAT_lo", bufs=2)
                nc.sync.dma_start(
                    out=pAT_lo,
                    in_=projATl_d[:].rearrange("(c p) l -> p c l", p=128)[
                        :, :, i * 128:(i + 1) * 128
                    ],
                )
                smask = p1.tile([128, L], F32, tag="smask", bufs=2)
                for h in range(2):
                    sps = p1_ps.tile([128, 1024], F32, tag="sps", bufs=2)
                    for nb in range(2):
                        msl = slice(h * 1024 + nb * 512, h * 1024 + (nb + 1) * 512)
                        terms = [(pAT_hi, bt_hi), (pAT_hi, bt_lo), (pAT_lo, bt_hi)]
                        for t, (lt, rt) in enumerate(terms):
                            for ec in range(KC):
                                nc.tensor.matmul(
                                    sps[:, nb * 512:(nb + 1) * 512],
                                    lt[:, ec, :],
                                    rt[:, ec, msl],
                                    start=(t == 0 and ec == 0),
                                    stop=(t == 2 and ec == KC - 1),
                                )
                    nc.vector.tensor_tensor(
                        out=smask[:, h * 1024:(h + 1) * 1024],
                        in0=sps,
                        in1=MBb[:, h * 1024:(h + 1) * 1024],
                        op=MIN,
                    )
                negrm = p1.tile([128, 1], F32, tag="negrm", bufs=2)
                nc.vector.reduce_max(out=negrm, in_=smask, axis=X, negate=True)
                biasB = p1.tile([128, 1], F32, tag="biasB", bufs=2)
                nc.vector.tensor_tensor(
                    out=biasB, in0=negrm, in1=maA[:, i:i + 1], op=MULT
                )
                eb = p1.tile([128, L], BF16, tag="eb", bufs=2)
                denomB = p1.tile([128, 1], F32, tag="denomB", bufs=2)
                nc.scalar.activation(
                    out=eb, in_=smask, func=EXP,
                    bias=biasB, scale=maA[:, i:i + 1], accum_out=denomB,
                )
                # fully-masked scores (A-mask applied too) -> DRAM for phase C;
                # overwrites smask in place after the E_B exp has consumed it
                nc.vector.tensor_scalar_min(smask, smask, maA_min[:, i:i + 1])
                nc.sync.dma_start(out=TMIN_d[i * 128:(i + 1) * 128, :], in_=smask)
                # per-strip column max -> running colmax (idle gpsimd engine)
                for h in range(2):
                    ar = p1.tile([128, 1024], F32, tag="ar", bufs=1)
                    hsl = slice(h * 1024, (h + 1) * 1024)
                    nc.gpsimd.partition_all_reduce(
                        ar, smask[:, hsl], channels=128,
                        reduce_op=bass_isa.ReduceOp.max,
                    )
                    nc.vector.tensor_tensor(
                        out=cmax_full[:, hsl], in0=cmax_full[:, hsl], in1=ar, op=MAXOP
                    )


                prev[i] = (eb, denomB)

            def stage2(i):
                eb, denomB = prev.pop(i)
                # E_B^T tiles and cvB strip
                ebt = p1.tile([128, NS, 128], BF16, tag="ebt", bufs=2)
                for g in range(2):
                    tp3 = p1_ps.tile([128, 8, 128], BF16, tag="tp", bufs=2)
                    for j in range(8):
                        mc = g * 8 + j
                        nc.tensor.transpose(
                            tp3[:, j, :], eb[:, mc * 128:(mc + 1) * 128], ident_bf
                        )
                    nc.scalar.copy(out=ebt[:, g * 8:(g + 1) * 8, :], in_=tp3)
                ups = p1_ps.tile([128, D], F32, tag="ups", bufs=1)
                for nb in range(2):
                    for mc in range(NS):
                        nc.tensor.matmul(
                            ups[:, nb * 512:(nb + 1) * 512],
                            ebt[:, mc, :],
                            b_bf[:, mc, nb * 512:(nb + 1) * 512],
                            start=(mc == 0),
                            stop=(mc == NS - 1),
                        )
                rden = p1.tile([128, 1], F32, tag="rden", bufs=2)
                nc.vector.reciprocal(out=rden, in_=denomB)
                cvb_sb = p1.tile([128, D], F32, tag="cvb_sb", bufs=2)
                nc.vector.tensor_scalar(
                    out=cvb_sb, in0=ups, scalar1=rden, scalar2=None, op0=MULT
                )
                nc.sync.dma_start(out=cvB[i * 128:(i + 1) * 128, :], in_=cvb_sb)


            for i in range(NS):
                stage1(i)
                if i > 0:
                    stage2(i - 1)
            stage2(NS - 1)

        # ---------------- Phase C: cvA per m-chunk ----------------
        with tc.tile_pool(name="pc", bufs=1) as pc, \
             tc.tile_pool(name="pc_ps", bufs=1, space="PSUM") as pc_ps:
            cprev = {}
            subs_last = {}

            def c_stage1(j):
                # tmin column slab: [l-part, lc, m] for 128 columns m
                slab = pc.tile([128, NS, 128], F32, tag="slab", bufs=6)
                nc.sync.dma_start(
                    out=slab,
                    in_=TMIN_d[:, j * 128:(j + 1) * 128].rearrange(
                        "(c p) m -> p c m", p=128
                    ),
                )
                # E_A tiles = exp(tmin - colmax), bf16 (colmax from pass 1)
                cmb = cmax_full[:, j * 128:(j + 1) * 128].unsqueeze(1).broadcast_to(
                    (128, 4, 128)
                )
                for q in range(4):
                    sub_inst = nc.vector.tensor_tensor(
                        out=slab[:, q * 4:(q + 1) * 4, :],
                        in0=slab[:, q * 4:(q + 1) * 4, :], in1=cmb, op=SUB,
                    )
                subs_last[j] = sub_inst
                ea_t = pc.tile([128, NS, 128], BF16, tag="ea_t", bufs=6)
                for hh in range(4):
                    nc.scalar.activation(
                        out=ea_t[:, hh * 4:(hh + 1) * 4, :],
                        in_=slab[:, hh * 4:(hh + 1) * 4, :], func=EXP,
                    )

                cprev[j] = ea_t

            def c_stage2(j):
                ea_t = cprev.pop(j)
                aps = pc_ps.tile([128, D], F32, tag="aps", bufs=3)
                csum = pc_ps.tile([128, 1], F32, tag="csum", bufs=2)
                for lc in range(NS):
                    nc.tensor.matmul(
                        csum, ea_t[:, lc, :], ones_bf,
                        start=(lc == 0), stop=(lc == NS - 1),
                    )
                for nb in range(2):
                    for lc in range(NS):
                        nc.tensor.matmul(
                            aps[:, nb * 512:(nb + 1) * 512],
                            ea_t[:, lc, :],
                            a_bf[:, lc, nb * 512:(nb + 1) * 512],
                            start=(lc == 0),
                            stop=(lc == NS - 1),
                        )
                rcs = pc.tile([128, 1], F32, tag="rcs", bufs=2)
                recip_inst = nc.vector.reciprocal(out=rcs, in_=csum)
                if j + 2 in subs_last:
                    tile.add_dep_helper(
                        recip_inst.ins, subs_last[j + 2].ins, sync=False,
                        reason="keep cvA evict after next chunks' subs on DVE",
                    )
                cva_sb = pc.tile([128, D], F32, tag="cva_sb", bufs=2)
                nc.vector.tensor_scalar(
                    out=cva_sb, in0=aps, scalar1=rcs, scalar2=None, op0=MULT
                )
                nc.sync.dma_start(out=cvA[j * 128:(j + 1) * 128, :], in_=cva_sb)


            c_stage1(0)
            c_stage1(1)
            for j in range(NS):
                if j + 2 < NS:
                    c_stage1(j + 2)
                c_stage2(j)
    if not nc.is_finalized():
        nc.finalize()
    return nc


def run(inputs, trace=False, trace_kwargs=None):
    if "nc" not in _CACHE:
        _CACHE["nc"] = build()
    nc = _CACHE["nc"]
    in_maps = []
    for i in range(B):
        in_maps.append({
            "inputA": np.ascontiguousarray(inputs["inputA"][i], dtype=np.float32),
            "inputB": np.ascontiguousarray(inputs["inputB"][i], dtype=np.float32),
            "maskA": np.ascontiguousarray(
                inputs["maskA"][i], dtype=np.float32).reshape(L, 1),
            "maskB": np.ascontiguousarray(
                inputs["maskB"][i], dtype=np.float32).reshape(1, L),
            "W": np.ascontiguousarray(inputs["W"], dtype=np.float32),
            "b": np.ascontiguousarray(inputs["b"], dtype=np.float32).reshape(D, 1),
        })
    try:
        res = run_bass_kernel_spmd(
            nc, in_maps, core_ids=list(range(B)), trace=trace,
            **(trace_kwargs or {}),
        )
    except ModuleNotFoundError:
        res = run_bass_kernel_spmd(nc, in_maps, core_ids=list(range(B)), trace=False)
    cva = np.stack([res.results[i]["cvA"] for i in range(B)]).astype(np.float32)
    cvb = np.stack([res.results[i]["cvB"] for i in range(B)]).astype(np.float32)
    return (cva, cvb), res


def kernel(**inputs):
    (cva, cvb), _ = run(inputs, trace=False)
    return cva, cvb
